# revision 3
# baseline (speedup 1.0000x reference)
"""Trainium2 Bass kernel for nn_DiscAdvLossForTarget_min.

Math (per batch row, x = logits[0:1000], e = extra logit x[1000]):
    prob_i = softmax(x)_i                  = exp(x_i - e) / sum_j exp(x_j - e)
    log pc_i = log sigmoid(e - x_i)        = -log1p(exp(x_i - e))
    loss = -(1/B) * sum_b sum_i prob_i * log(pc_i)
         = +(1/B) * sum_b U_b / S_b
    where a_i = exp(x_i - e), U_b = sum_i a_i * log1p(a_i), S_b = sum_i a_i.

Device mapping (per core, data-parallel over batch, 8192 rows per core,
64 row-blocks of 128). Work is emitted in supertiles of g blocks (one
input DMA each). Per supertile k:
  ACT: one batched Exp over the g*1000 class logits -> t (bf16), and a
       tiny batched Exp(scale=-1) over the g extra-logit columns
       -> c = exp(-e) (f32, per row).
  ACT: batched Ln(a+1) of the PREVIOUS supertile (one-supertile software
       pipelining so ACT never waits on the DVE scales of k).
  DVE: per block, tensor_scalar a = t * c with accum_out -> S column
       (runs in 4x mode: bf16 operands, per-partition f32 scalar), then
       per block of k-1 a scalar_tensor_tensor (a * 1) * w with
       accum_out -> U column (1x; this is the DVE's main cost).
ACT is the bottleneck engine (two table passes over every element,
~0.83ns/elem); everything else hides behind it. Exp and Ln resolve to
one resident table set via _PinnedBacc, so there is a single
ACT_TABLE_LOAD.
Host: loss = (1/B) * sum over rows/cores of U/S.
"""

import numpy as np

import bass_rust as _bass_rust
import concourse.bacc as bacc
import concourse.bass as bass
import concourse.tile as tile
from concourse import bass_utils, mybir
from concourse.hw_specs import get_activation_tables

N_CORES = 8
B_FULL = 65536
C1 = 1001
C = 1000
P = 128
B_SHARD = B_FULL // N_CORES  # 8192
N_BLOCKS = B_SHARD // P  # 64
G_MAX = 8

# supertile sizes (sum = N_BLOCKS); small head so the first Exp starts as
# soon as one block of data lands, small tail to shorten the drain.
PLAN = [1, 2, 4] + [8] * 7 + [1]
assert sum(PLAN) == N_BLOCKS


class _PinnedBacc(bacc.Bacc):
    """Bacc whose activation-table chooser only sees sets containing every
    activation function this kernel uses, so Exp and Ln resolve to one
    resident set (natural_log_exp_and_others) instead of thrashing
    ACT_TABLE_LOADs between per-function sets."""

    def insert_act_table_loads(self):
        used = {
            i.func
            for b in self.main_func.blocks
            for i in b.instructions
            if isinstance(i, mybir.InstActivation)
        }
        if not used:
            return
        tables = [
            (name, fns if used <= fns else set())
            for name, fns in get_activation_tables(self.m.arch).items()
        ]
        _bass_rust.insert_act_table_loads(self, tables)


_nc_cache = None


def _build() -> bass.Bass:
    global _nc_cache
    if _nc_cache is not None:
        return _nc_cache

    nc = _PinnedBacc("TRN2", debug=False)
    x = nc.dram_tensor("x", [B_SHARD, C1], mybir.dt.float32, kind="ExternalInput").ap()
    u_out = nc.dram_tensor(
        "u_out", [P, N_BLOCKS], mybir.dt.float32, kind="ExternalOutput"
    ).ap()
    s_out = nc.dram_tensor(
        "s_out", [P, N_BLOCKS], mybir.dt.float32, kind="ExternalOutput"
    ).ap()

    # Shard row handled by (partition p, block n): row = p*N_BLOCKS + n, so a
    # run of consecutive blocks is contiguous DRAM per partition.
    x_r = x.rearrange("(p n) m -> p n m", p=P, n=N_BLOCKS)

    with tile.TileContext(nc) as tc:
        with (
            tc.tile_pool(name="xin", bufs=2) as xin,
            tc.tile_pool(name="mid", bufs=2) as mid,
            tc.tile_pool(name="small", bufs=3) as small,
            tc.tile_pool(name="accp", bufs=1) as accp,
        ):
            U = accp.tile([P, N_BLOCKS], mybir.dt.float32)
            S = accp.tile([P, N_BLOCKS], mybir.dt.float32)
            n0 = 0  # first block of the current supertile
            prev = None  # (n0, g, aa, ww) of the previous supertile
            for g in PLAN:
                xt = xin.tile([P, G_MAX, C1], mybir.dt.float32, tag="xt")
                nc.sync.dma_start(out=xt[:, 0:g, :], in_=x_r[:, n0 : n0 + g, :])

                # ACT: batched Exp of the class logits only (3D strided AP
                # skips the extra-logit column), then exp(-e) for the rows.
                tt = mid.tile([P, G_MAX, C], mybir.dt.bfloat16, tag="tt")
                nc.scalar.activation(
                    out=tt[:, 0:g, :],
                    in_=xt[:, 0:g, 0:C],
                    func=mybir.ActivationFunctionType.Exp,
                )
                cc = small.tile([P, G_MAX], mybir.dt.float32, tag="cc")
                nc.scalar.activation(
                    out=cc[:, 0:g],
                    in_=xt[:, 0:g, C],
                    func=mybir.ActivationFunctionType.Exp,
                    scale=-1.0,
                )

                # ACT: Ln of the previous supertile (pipelined one behind so
                # the DVE scales below have a full Exp batch to hide under).
                ww = None
                if prev is not None:
                    pn0, pg, paa, pww = prev
                    nc.scalar.activation(
                        out=pww[:, 0:pg, :].rearrange("p g c -> p (g c)"),
                        in_=paa[:, 0:pg, :].rearrange("p g c -> p (g c)"),
                        func=mybir.ActivationFunctionType.Ln,
                        bias=1.0,
                        scale=1.0,
                    )

                # DVE: a = t * exp(-e) per block, 4x mode, S rides accum_out.
                aa = mid.tile([P, G_MAX, C], mybir.dt.bfloat16, tag="aa")
                for i in range(g):
                    col = n0 + i
                    nc.vector.tensor_scalar(
                        out=aa[:, i, :],
                        in0=tt[:, i, :],
                        scalar1=cc[:, i : i + 1],
                        scalar2=0.0,
                        op0=mybir.AluOpType.mult,
                        op1=mybir.AluOpType.add,
                        accum_out=S[:, col : col + 1],
                    )

                # DVE: U columns of the previous supertile (its Ln is above).
                if prev is not None:
                    pn0, pg, paa, pww = prev
                    for i in range(pg):
                        col = pn0 + i
                        scr = mid.tile([P, C], mybir.dt.bfloat16, tag="scr")
                        nc.vector.scalar_tensor_tensor(
                            out=scr,
                            in0=paa[:, i, :],
                            scalar=1.0,
                            in1=pww[:, i, :],
                            op0=mybir.AluOpType.mult,
                            op1=mybir.AluOpType.mult,
                            accum_out=U[:, col : col + 1],
                        )

                ww = mid.tile([P, G_MAX, C], mybir.dt.bfloat16, tag="ww")
                prev = (n0, g, aa, ww)
                n0 += g

            # drain: Ln + U of the last supertile
            pn0, pg, paa, pww = prev
            nc.scalar.activation(
                out=pww[:, 0:pg, :].rearrange("p g c -> p (g c)"),
                in_=paa[:, 0:pg, :].rearrange("p g c -> p (g c)"),
                func=mybir.ActivationFunctionType.Ln,
                bias=1.0,
                scale=1.0,
            )
            for i in range(pg):
                col = pn0 + i
                scr = mid.tile([P, C], mybir.dt.bfloat16, tag="scr")
                nc.vector.scalar_tensor_tensor(
                    out=scr,
                    in0=paa[:, i, :],
                    scalar=1.0,
                    in1=pww[:, i, :],
                    op0=mybir.AluOpType.mult,
                    op1=mybir.AluOpType.mult,
                    accum_out=U[:, col : col + 1],
                )

            nc.sync.dma_start(out=u_out, in_=U)
            nc.sync.dma_start(out=s_out, in_=S)

    nc.finalize()  # runs Bacc passes (wait splitting, reg alloc, ...)
    _nc_cache = nc
    return nc


LAST_RESULTS = None


def kernel(input: np.ndarray, target: np.ndarray | None = None, _trace: bool = False, **_unused) -> np.ndarray:
    global LAST_RESULTS
    input = np.ascontiguousarray(np.asarray(input, dtype=np.float32))
    assert input.shape == (B_FULL, C1), input.shape

    nc = _build()
    in_maps = [
        {"x": input[i * B_SHARD : (i + 1) * B_SHARD]} for i in range(N_CORES)
    ]
    res = bass_utils.run_bass_kernel_spmd(
        nc, in_maps, core_ids=list(range(N_CORES)), trace=_trace
    )
    LAST_RESULTS = res
    total = np.float64(0.0)
    for r in res.results:
        u = np.asarray(r["u_out"], dtype=np.float64)
        s = np.asarray(r["s_out"], dtype=np.float64)
        total += (u / s).sum()
    # w = log1p(a) = -log(pc) already carries the loss's minus sign.
    loss = total / B_FULL
    return np.float32(loss)


# revision 5
# speedup vs baseline: 1.1631x; 1.1631x over previous
"""Trainium2 Bass kernel for nn_DiscAdvLossForTarget_min.

Math (per batch row, x = logits[0:1000], e = extra logit x[1000]):
    prob_i = softmax(x)_i                  = exp(x_i - e) / sum_j exp(x_j - e)
    log pc_i = log sigmoid(e - x_i)        = -log1p(exp(x_i - e))
    loss = -(1/B) * sum_b sum_i prob_i * log(pc_i)
         = +(1/B) * sum_b U_b / S_b
    where a_i = exp(x_i - e), U_b = sum_i a_i * log1p(a_i), S_b = sum_i a_i.

Device mapping (per core, data-parallel over batch, 8192 rows per core,
64 row-blocks of 128). Work is emitted in supertiles of g blocks (one
input DMA each). HW-measured per-block costs (128x1000, bf16
intermediates): ACT batched act 857ns, ACT per-block act w/ bias +
accum read 1205ns, DVE tensor_scalar a=t*c w/ accum_out 1203ns (the
accum variant runs 1x on silicon even though the cost model says 4x),
DVE scalar_tensor_tensor (U product w/ accum) 1112ns.  The balanced
assignment (ACT ~= DVE ~= 118us busy) is ka mode-A blocks per
supertile (ACT: Exp(x + bias(-e)) with accum_out -> S, no DVE work)
and the rest mode D (ACT: batched Exp -> t; DVE: one 1x
tensor_scalar fold a = t*exp(-e) with accum_out -> S).
Then one batched ACT Ln(a+1) -> w per supertile and per-block DVE
scalar_tensor_tensor (a*1)*w with accum_out -> U.

Pipelining: Ln of supertile k-1 is emitted inside supertile k, between
the mode-D Exp and the mode-A Exps, so ACT never waits on the DVE
folds and the DVE always has a full supertile of stt work queued.  A
dummy 1-element activation at the top hoists the single
ACT_TABLE_LOAD (Exp and Ln share one table set via _PinnedBacc) into
the DMA fill window.
Host: loss = (1/B) * sum over rows/cores of U/S.
"""

import numpy as np

import bass_rust as _bass_rust
import concourse.bacc as bacc
import concourse.bass as bass
import concourse.tile as tile
from concourse import bass_utils, mybir
from concourse.hw_specs import get_activation_tables

N_CORES = 8
B_FULL = 65536
C1 = 1001
C = 1000
P = 128
B_SHARD = B_FULL // N_CORES  # 8192
N_BLOCKS = B_SHARD // P  # 64
G_MAX = 8

# supertile sizes (sum = N_BLOCKS); small head so the first Exp starts as
# soon as one block of data lands, small tail to shorten the drain.
PLAN = [1, 2, 4] + [8] * 7 + [1]
assert sum(PLAN) == N_BLOCKS
# mode-A blocks per supertile (ACT-side S) vs mode-D (DVE-side S fold).
KA_OF = {1: 0, 2: 1, 4: 2, 8: 3}


class _PinnedBacc(bacc.Bacc):
    """Bacc whose activation-table chooser only sees sets containing every
    activation function this kernel uses, so Exp and Ln resolve to one
    resident set (natural_log_exp_and_others) instead of thrashing
    ACT_TABLE_LOADs between per-function sets."""

    def insert_act_table_loads(self):
        used = {
            i.func
            for b in self.main_func.blocks
            for i in b.instructions
            if isinstance(i, mybir.InstActivation)
        }
        if not used:
            return
        tables = [
            (name, fns if used <= fns else set())
            for name, fns in get_activation_tables(self.m.arch).items()
        ]
        _bass_rust.insert_act_table_loads(self, tables)


_nc_cache = None


def _build() -> bass.Bass:
    global _nc_cache
    if _nc_cache is not None:
        return _nc_cache

    nc = _PinnedBacc("TRN2", debug=False)
    x = nc.dram_tensor("x", [B_SHARD, C1], mybir.dt.float32, kind="ExternalInput").ap()
    u_out = nc.dram_tensor(
        "u_out", [P, N_BLOCKS], mybir.dt.float32, kind="ExternalOutput"
    ).ap()
    s_out = nc.dram_tensor(
        "s_out", [P, N_BLOCKS], mybir.dt.float32, kind="ExternalOutput"
    ).ap()

    # Shard row handled by (partition p, block n): row = p*N_BLOCKS + n, so a
    # run of consecutive blocks is contiguous DRAM per partition.
    x_r = x.rearrange("(p n) m -> p n m", p=P, n=N_BLOCKS)

    with tile.TileContext(nc) as tc:
        with (
            tc.tile_pool(name="xin", bufs=2) as xin,
            tc.tile_pool(name="mid", bufs=2) as mid,
            tc.tile_pool(name="small", bufs=3) as small,
            tc.tile_pool(name="accp", bufs=1) as accp,
        ):
            U = accp.tile([P, N_BLOCKS], mybir.dt.float32)
            S = accp.tile([P, N_BLOCKS], mybir.dt.float32)

            # Dummy activation on a constant tile: pulls the ACT_TABLE_LOAD
            # into the DMA fill window instead of after the first data lands.
            warm = accp.tile([P, 2], mybir.dt.float32)
            nc.vector.memset(warm, 0.0)
            nc.scalar.activation(
                out=warm[:, 1:2],
                in_=warm[:, 0:1],
                func=mybir.ActivationFunctionType.Exp,
            )

            n0 = 0  # first block of the current supertile
            prev = None  # (n0, g, aa, ww) of the previous supertile
            for g in PLAN:
                ka = KA_OF[g]
                kd = g - ka
                xt = xin.tile([P, G_MAX, C1], mybir.dt.float32, tag="xt")
                nc.sync.dma_start(out=xt[:, 0:g, :], in_=x_r[:, n0 : n0 + g, :])

                aa = mid.tile([P, G_MAX, C], mybir.dt.bfloat16, tag="aa")

                # mode D: batched Exp of the class logits (3D strided AP
                # skips the extra-logit column) -> t, plus exp(-e) per row.
                if kd:
                    tt = mid.tile([P, G_MAX, C], mybir.dt.bfloat16, tag="tt")
                    nc.scalar.activation(
                        out=tt[:, 0:kd, :],
                        in_=xt[:, ka:g, 0:C],
                        func=mybir.ActivationFunctionType.Exp,
                    )
                    cc = small.tile([P, G_MAX], mybir.dt.float32, tag="cc")
                    nc.scalar.activation(
                        out=cc[:, 0:kd],
                        in_=xt[:, ka:g, C],
                        func=mybir.ActivationFunctionType.Exp,
                        scale=-1.0,
                    )

                # ACT: Ln of the previous supertile (pipelined one behind so
                # the DVE folds of k-1 are long done and ACT never stalls).
                if prev is not None:
                    pn0, pg, paa, pww = prev
                    nc.scalar.activation(
                        out=pww[:, 0:pg, :].rearrange("p g c -> p (g c)"),
                        in_=paa[:, 0:pg, :].rearrange("p g c -> p (g c)"),
                        func=mybir.ActivationFunctionType.Ln,
                        bias=1.0,
                        scale=1.0,
                    )

                # mode A: per-block Exp with bias(-e), accum_out -> S col.
                if ka:
                    neg_e = small.tile([P, G_MAX], mybir.dt.float32, tag="neg_e")
                    nc.vector.tensor_scalar_mul(neg_e[:, 0:ka], xt[:, 0:ka, C], -1.0)
                    for i in range(ka):
                        col = n0 + i
                        nc.scalar.activation(
                            out=aa[:, i, :],
                            in_=xt[:, i, 0:C],
                            func=mybir.ActivationFunctionType.Exp,
                            bias=neg_e[:, i : i + 1],
                            scale=1.0,
                            accum_out=S[:, col : col + 1],
                        )

                # DVE: mode-D fold a = t * exp(-e), accum_out -> S col (1x).
                for j in range(kd):
                    col = n0 + ka + j
                    nc.vector.tensor_scalar(
                        out=aa[:, ka + j, :],
                        in0=tt[:, j, :],
                        scalar1=cc[:, j : j + 1],
                        scalar2=0.0,
                        op0=mybir.AluOpType.mult,
                        op1=mybir.AluOpType.add,
                        accum_out=S[:, col : col + 1],
                    )

                # DVE: U columns of the previous supertile.
                if prev is not None:
                    pn0, pg, paa, pww = prev
                    for i in range(pg):
                        col = pn0 + i
                        scr = mid.tile([P, C], mybir.dt.bfloat16, tag="scr")
                        nc.vector.scalar_tensor_tensor(
                            out=scr,
                            in0=paa[:, i, :],
                            scalar=1.0,
                            in1=pww[:, i, :],
                            op0=mybir.AluOpType.mult,
                            op1=mybir.AluOpType.mult,
                            accum_out=U[:, col : col + 1],
                        )

                ww = mid.tile([P, G_MAX, C], mybir.dt.bfloat16, tag="ww")
                prev = (n0, g, aa, ww)
                n0 += g

            # drain: Ln + U of the last supertile
            pn0, pg, paa, pww = prev
            nc.scalar.activation(
                out=pww[:, 0:pg, :].rearrange("p g c -> p (g c)"),
                in_=paa[:, 0:pg, :].rearrange("p g c -> p (g c)"),
                func=mybir.ActivationFunctionType.Ln,
                bias=1.0,
                scale=1.0,
            )
            for i in range(pg):
                col = pn0 + i
                scr = mid.tile([P, C], mybir.dt.bfloat16, tag="scr")
                nc.vector.scalar_tensor_tensor(
                    out=scr,
                    in0=paa[:, i, :],
                    scalar=1.0,
                    in1=pww[:, i, :],
                    op0=mybir.AluOpType.mult,
                    op1=mybir.AluOpType.mult,
                    accum_out=U[:, col : col + 1],
                )

            nc.sync.dma_start(out=u_out, in_=U)
            nc.sync.dma_start(out=s_out, in_=S)

    nc.finalize()  # runs Bacc passes (wait splitting, reg alloc, ...)
    _nc_cache = nc
    return nc


LAST_RESULTS = None


def kernel(input: np.ndarray, target: np.ndarray | None = None, _trace: bool = False, **_unused) -> np.ndarray:
    global LAST_RESULTS
    input = np.ascontiguousarray(np.asarray(input, dtype=np.float32))
    assert input.shape == (B_FULL, C1), input.shape

    nc = _build()
    in_maps = [
        {"x": input[i * B_SHARD : (i + 1) * B_SHARD]} for i in range(N_CORES)
    ]
    res = bass_utils.run_bass_kernel_spmd(
        nc, in_maps, core_ids=list(range(N_CORES)), trace=_trace
    )
    LAST_RESULTS = res
    total = np.float64(0.0)
    for r in res.results:
        u = np.asarray(r["u_out"], dtype=np.float64)
        s = np.asarray(r["s_out"], dtype=np.float64)
        total += (u / s).sum()
    # w = log1p(a) = -log(pc) already carries the loss's minus sign.
    loss = total / B_FULL
    return np.float32(loss)


# revision 7
# speedup vs baseline: 1.1851x; 1.0189x over previous
"""Trainium2 Bass kernel for nn_DiscAdvLossForTarget_min.

Math (per batch row, x = logits[0:1000], e = extra logit x[1000]):
    prob_i = softmax(x)_i                  = exp(x_i - e) / sum_j exp(x_j - e)
    log pc_i = log sigmoid(e - x_i)        = -log1p(exp(x_i - e))
    loss = -(1/B) * sum_b sum_i prob_i * log(pc_i)
         = +(1/B) * sum_b U_b / S_b
    where a_i = exp(x_i - e), U_b = sum_i a_i * log1p(a_i), S_b = sum_i a_i.

Device mapping (per core, data-parallel over batch, 8192 rows per core,
64 row-blocks of 128). Work is emitted in supertiles of g blocks (one
input DMA each). HW-measured per-block costs (128x1000, bf16
intermediates): ACT batched act 857ns, ACT per-block act w/ bias +
accum read 1205ns, DVE tensor_scalar a=t*c w/ accum_out 1203ns (the
accum variant runs 1x on silicon even though the cost model says 4x),
DVE scalar_tensor_tensor (U product w/ accum) 1112ns.  The balanced
assignment (ACT ~= DVE ~= 118us busy) is ka mode-A blocks per
supertile (ACT: Exp(x + bias(-e)) with accum_out -> S, no DVE work)
and the rest mode D (ACT: batched Exp -> t; DVE: one 1x
tensor_scalar fold a = t*exp(-e) with accum_out -> S).
Then one batched ACT Ln(a+1) -> w per supertile and per-block DVE
scalar_tensor_tensor (a*1)*w with accum_out -> U.

Pipelining: Ln of supertile k-1 is emitted inside supertile k, between
the mode-D Exp and the mode-A Exps, so ACT never waits on the DVE
folds and the DVE always has a full supertile of stt work queued.  A
dummy 1-element activation at the top hoists the single
ACT_TABLE_LOAD (Exp and Ln share one table set via _PinnedBacc) into
the DMA fill window.
Host: loss = (1/B) * sum over rows/cores of U/S.
"""

import numpy as np

import bass_rust as _bass_rust
import concourse.bacc as bacc
import concourse.bass as bass
import concourse.tile as tile
from concourse import bass_utils, mybir
from concourse.hw_specs import get_activation_tables

N_CORES = 8
B_FULL = 65536
C1 = 1001
C = 1000
P = 128
B_SHARD = B_FULL // N_CORES  # 8192
N_BLOCKS = B_SHARD // P  # 64
G_MAX = 8

# supertile sizes (sum = N_BLOCKS); small head so the first Exp starts as
# soon as one block of data lands, descending tail so the one-supertile
# pipelined Ln/stt drain stays short.
PLAN = [1, 2, 4, 8, 8, 8, 8, 8, 8, 5, 3, 1]
assert sum(PLAN) == N_BLOCKS
# mode-A blocks per supertile (ACT-side S) vs mode-D (DVE-side S fold).
KA_OF = {1: 0, 2: 1, 3: 1, 4: 2, 5: 2, 8: 3}


class _PinnedBacc(bacc.Bacc):
    """Bacc whose activation-table chooser only sees sets containing every
    activation function this kernel uses, so Exp and Ln resolve to one
    resident set (natural_log_exp_and_others) instead of thrashing
    ACT_TABLE_LOADs between per-function sets."""

    def insert_act_table_loads(self):
        used = {
            i.func
            for b in self.main_func.blocks
            for i in b.instructions
            if isinstance(i, mybir.InstActivation)
        }
        if not used:
            return
        tables = [
            (name, fns if used <= fns else set())
            for name, fns in get_activation_tables(self.m.arch).items()
        ]
        _bass_rust.insert_act_table_loads(self, tables)


_nc_cache = None


def _build() -> bass.Bass:
    global _nc_cache
    if _nc_cache is not None:
        return _nc_cache

    nc = _PinnedBacc("TRN2", debug=False)
    x = nc.dram_tensor("x", [B_SHARD, C1], mybir.dt.float32, kind="ExternalInput").ap()
    u_out = nc.dram_tensor(
        "u_out", [P, N_BLOCKS], mybir.dt.float32, kind="ExternalOutput"
    ).ap()
    s_out = nc.dram_tensor(
        "s_out", [P, N_BLOCKS], mybir.dt.float32, kind="ExternalOutput"
    ).ap()

    # Shard row handled by (partition p, block n): row = p*N_BLOCKS + n, so a
    # run of consecutive blocks is contiguous DRAM per partition.
    x_r = x.rearrange("(p n) m -> p n m", p=P, n=N_BLOCKS)

    with tile.TileContext(nc) as tc:
        with (
            tc.tile_pool(name="xin", bufs=3) as xin,
            tc.tile_pool(name="mid", bufs=2) as mid,
            tc.tile_pool(name="small", bufs=3) as small,
            tc.tile_pool(name="accp", bufs=1) as accp,
        ):
            U = accp.tile([P, N_BLOCKS], mybir.dt.float32)
            S = accp.tile([P, N_BLOCKS], mybir.dt.float32)

            # Dummy activation on a constant tile: pulls the ACT_TABLE_LOAD
            # into the DMA fill window instead of after the first data lands.
            warm = accp.tile([P, 2], mybir.dt.float32)
            nc.vector.memset(warm, 0.0)
            nc.scalar.activation(
                out=warm[:, 1:2],
                in_=warm[:, 0:1],
                func=mybir.ActivationFunctionType.Exp,
            )

            n0 = 0  # first block of the current supertile
            prev = None  # (n0, g, aa, ww) of the previous supertile
            for g in PLAN:
                ka = KA_OF[g]
                kd = g - ka
                xt = xin.tile([P, G_MAX, C1], mybir.dt.float32, tag="xt")
                nc.sync.dma_start(out=xt[:, 0:g, :], in_=x_r[:, n0 : n0 + g, :])

                aa = mid.tile([P, G_MAX, C], mybir.dt.bfloat16, tag="aa")

                # mode D: batched Exp of the class logits (3D strided AP
                # skips the extra-logit column) -> t, plus exp(-e) per row.
                if kd:
                    tt = mid.tile([P, G_MAX, C], mybir.dt.bfloat16, tag="tt")
                    nc.scalar.activation(
                        out=tt[:, 0:kd, :],
                        in_=xt[:, ka:g, 0:C],
                        func=mybir.ActivationFunctionType.Exp,
                    )
                    cc = small.tile([P, G_MAX], mybir.dt.float32, tag="cc")
                    nc.scalar.activation(
                        out=cc[:, 0:kd],
                        in_=xt[:, ka:g, C],
                        func=mybir.ActivationFunctionType.Exp,
                        scale=-1.0,
                    )

                # ACT: Ln of the previous supertile (pipelined one behind so
                # the DVE folds of k-1 are long done and ACT never stalls).
                if prev is not None:
                    pn0, pg, paa, pww = prev
                    nc.scalar.activation(
                        out=pww[:, 0:pg, :].rearrange("p g c -> p (g c)"),
                        in_=paa[:, 0:pg, :].rearrange("p g c -> p (g c)"),
                        func=mybir.ActivationFunctionType.Ln,
                        bias=1.0,
                        scale=1.0,
                    )

                # mode A: per-block Exp with bias(-e), accum_out -> S col.
                if ka:
                    neg_e = small.tile([P, G_MAX], mybir.dt.float32, tag="neg_e")
                    nc.vector.tensor_scalar_mul(neg_e[:, 0:ka], xt[:, 0:ka, C], -1.0)
                    for i in range(ka):
                        col = n0 + i
                        nc.scalar.activation(
                            out=aa[:, i, :],
                            in_=xt[:, i, 0:C],
                            func=mybir.ActivationFunctionType.Exp,
                            bias=neg_e[:, i : i + 1],
                            scale=1.0,
                            accum_out=S[:, col : col + 1],
                        )

                # DVE: mode-D fold a = t * exp(-e), accum_out -> S col (1x).
                for j in range(kd):
                    col = n0 + ka + j
                    nc.vector.tensor_scalar(
                        out=aa[:, ka + j, :],
                        in0=tt[:, j, :],
                        scalar1=cc[:, j : j + 1],
                        scalar2=0.0,
                        op0=mybir.AluOpType.mult,
                        op1=mybir.AluOpType.add,
                        accum_out=S[:, col : col + 1],
                    )

                # DVE: U columns of the previous supertile.
                if prev is not None:
                    pn0, pg, paa, pww = prev
                    for i in range(pg):
                        col = pn0 + i
                        scr = mid.tile([P, C], mybir.dt.bfloat16, tag="scr")
                        nc.vector.scalar_tensor_tensor(
                            out=scr,
                            in0=paa[:, i, :],
                            scalar=1.0,
                            in1=pww[:, i, :],
                            op0=mybir.AluOpType.mult,
                            op1=mybir.AluOpType.mult,
                            accum_out=U[:, col : col + 1],
                        )

                ww = mid.tile([P, G_MAX, C], mybir.dt.bfloat16, tag="ww")
                prev = (n0, g, aa, ww)
                n0 += g

            # drain: Ln + U of the last supertile
            pn0, pg, paa, pww = prev
            nc.scalar.activation(
                out=pww[:, 0:pg, :].rearrange("p g c -> p (g c)"),
                in_=paa[:, 0:pg, :].rearrange("p g c -> p (g c)"),
                func=mybir.ActivationFunctionType.Ln,
                bias=1.0,
                scale=1.0,
            )
            for i in range(pg):
                col = pn0 + i
                scr = mid.tile([P, C], mybir.dt.bfloat16, tag="scr")
                nc.vector.scalar_tensor_tensor(
                    out=scr,
                    in0=paa[:, i, :],
                    scalar=1.0,
                    in1=pww[:, i, :],
                    op0=mybir.AluOpType.mult,
                    op1=mybir.AluOpType.mult,
                    accum_out=U[:, col : col + 1],
                )

            nc.sync.dma_start(out=u_out, in_=U)
            nc.sync.dma_start(out=s_out, in_=S)

    nc.finalize()  # runs Bacc passes (wait splitting, reg alloc, ...)
    _nc_cache = nc
    return nc


LAST_RESULTS = None


def kernel(input: np.ndarray, target: np.ndarray | None = None, _trace: bool = False, **_unused) -> np.ndarray:
    global LAST_RESULTS
    input = np.ascontiguousarray(np.asarray(input, dtype=np.float32))
    assert input.shape == (B_FULL, C1), input.shape

    nc = _build()
    in_maps = [
        {"x": input[i * B_SHARD : (i + 1) * B_SHARD]} for i in range(N_CORES)
    ]
    res = bass_utils.run_bass_kernel_spmd(
        nc, in_maps, core_ids=list(range(N_CORES)), trace=_trace
    )
    LAST_RESULTS = res
    total = np.float64(0.0)
    for r in res.results:
        u = np.asarray(r["u_out"], dtype=np.float64)
        s = np.asarray(r["s_out"], dtype=np.float64)
        total += (u / s).sum()
    # w = log1p(a) = -log(pc) already carries the loss's minus sign.
    loss = total / B_FULL
    return np.float32(loss)
